# revision 5
# baseline (speedup 1.0000x reference)
# Depthwise causal conv2d (N=2, C=16, H=W=2048, kernel 6x11) on 8 TRN2 cores.
#
# y[b,c,p,q] = sum_{r,s} w[c,r,s] * xm[b,c, p+r-5, q+s-5], xm = tril-masked x,
# y tril-masked.  Sharding: the 32 (b,c) images are independent; 4 per core.
#
# Per-core compute: for each output tile of [M<=123 rows, Nd<=512 cols], the
# row-tap contraction is a banded-Toeplitz matmul: for each of the S=11
# column taps, out[m, n] += band_s[k, m] * x[k, n+s] where band_s[k, m] =
# w[c, k-m, s] (k-m in [0,6)).  11 accumulating fp32r matmuls per tile into
# one PSUM bank.  Tiles fully above the causal diagonal are never computed or
# written (output DRAM is pre-zeroed); tiles crossing it get the causal mask
# applied to the input (gpsimd affine_select in SBUF) and to the output
# (DVE multiply with a staircase 0/1 tile during PSUM evacuation).
import sys

sys.path.insert(0, "/opt/trn_rl_repo")

import numpy as np

import concourse.bacc as bacc
import concourse.mybir as mybir
import concourse.tile as tile
from concourse.bass_utils import run_bass_kernel_spmd

N, C, H, W = 2, 16, 2048, 2048
R, S, PH, PW = 6, 11, 5, 5
NCORES = 8
IPC = (N * C) // NCORES  # images per core
MT = 123  # output rows per row-tile (128 input rows incl. 5-row halo)
NTS = 512  # max output cols per tile (one PSUM bank of fp32)
BANDW = 128  # allocated band width (cols used: M)
STAIR_W = 1536  # staircase mask width
STAIR_C = 511  # staircase offset: stair[i, u] = 1 iff u <= i + STAIR_C
F32R = mybir.dt.float32r
F32 = mybir.dt.float32

_NC_CACHE = {}


def _row_tiles():
    out = []
    p0 = 0
    while p0 < H:
        out.append((p0, min(MT, H - p0)))
        p0 += MT
    return out


def _col_tiles(pmax):
    """Column tiles covering q in [0, pmax]; width 512 except the last,
    which shrinks to a multiple of 128 (>=256 for full-rate fp32r)."""
    out = []
    q0 = 0
    while q0 <= pmax:
        needed = pmax - q0 + 1
        if needed >= NTS:
            nd = NTS
        else:
            nd = min(NTS, max(256, 128 * ((needed + 127) // 128)))
        out.append((q0, nd))
        q0 += nd
    return out


def _build_program():
    """One SPMD program: conv of IPC images [H, W] with per-image bands."""
    nc = bacc.Bacc("TRN2", target_bir_lowering=False, debug=False,
                   num_devices=NCORES)
    x = nc.dram_tensor("x", [IPC, H, W], F32R, kind="ExternalInput")
    bands = nc.dram_tensor("bands", [IPC, 128, S * BANDW], F32R,
                           kind="ExternalInput")
    y = nc.dram_tensor("y", [IPC, H, W], F32, kind="ExternalOutput")

    row_tiles = _row_tiles()

    with tile.TileContext(nc) as tc:
        with (
            tc.tile_pool(name="const", bufs=1) as cpool,
            tc.tile_pool(name="xin", bufs=4) as xpool,
            tc.tile_pool(name="out", bufs=4) as opool,
            tc.tile_pool(name="psum", bufs=6, space="PSUM") as ppool,
        ):
            # Per-image Toeplitz bands, resident for the whole kernel.
            bt = cpool.tile([128, IPC * S * BANDW], F32R)
            for i in range(IPC):
                nc.sync.dma_start(
                    out=bt[:, i * S * BANDW:(i + 1) * S * BANDW],
                    in_=bands[i],
                )
            # Staircase causal mask: stair[i, u] = 1 iff i + STAIR_C - u >= 0.
            stair = cpool.tile([128, STAIR_W], F32)
            nc.gpsimd.memset(stair[:], 1.0)
            nc.gpsimd.affine_select(
                out=stair[:], in_=stair[:],
                compare_op=mybir.AluOpType.is_ge, fill=0.0,
                base=STAIR_C, channel_multiplier=1,
                pattern=[[-1, STAIR_W]],
            )

            for i in range(IPC):
                band_i = bt[:, i * S * BANDW:(i + 1) * S * BANDW]
                for (p0, M) in row_tiles:
                    pmax = p0 + M - 1
                    for (q0, nd) in _col_tiles(pmax):
                        _emit_tile(nc, tc, xpool, opool, ppool, x, y, band_i,
                                   stair, i, p0, M, pmax, q0, nd)
    nc.compile()
    return nc


def _emit_tile(nc, tc, xpool, opool, ppool, x, y, band_i, stair,
               i, p0, M, pmax, q0, nd):
    inw = nd + S - 1  # input tile width incl. halo
    # Input rows [p0-PH, p0+M), cols [q0-PW, q0+nd+PW) clipped to the image
    # and (on the right) to the causal extent pmax.
    h0 = p0 - PH
    hv0, hv1 = max(0, h0), min(H, p0 + M)
    w0 = q0 - PW
    wv0 = max(0, w0)
    wv1 = min(W, q0 + nd + PW, pmax + 1)

    d = p0 - q0  # diagonal offset of this tile
    # Causal mask on input needed iff the loaded region touches h < w.
    in_mask = h0 < wv1 - 1
    # Causal mask on output needed iff the tile crosses p < q.
    out_mask = p0 < q0 + nd - 1

    xt = xpool.tile([128, inw], F32R, tag="xin")
    nc.sync.dma_start(
        out=xt[hv0 - h0:hv1 - h0, wv0 - w0:wv1 - w0],
        in_=x[i, hv0:hv1, wv0:wv1],
    )
    if q0 == 0:
        # w in [-5, 0) is zero padding that the causal select keeps
        # (memset can't write fp32r): keep iff j >= PW.
        nc.gpsimd.affine_select(
            out=xt[:], in_=xt[:],
            compare_op=mybir.AluOpType.is_ge, fill=0.0,
            base=-PW, channel_multiplier=0,
            pattern=[[1, inw]],
        )
    if in_mask:
        # keep iff (h0 + k) >= (w0 + j)  <=>  k - j + (h0 - w0) >= 0.
        # Also zeroes the unloaded right-of-pmax region and, for p0 == 0,
        # the k < PH halo rows above the image (h < 0 keeps nothing).
        nc.gpsimd.affine_select(
            out=xt[:], in_=xt[:],
            compare_op=mybir.AluOpType.is_ge, fill=0.0,
            base=h0 - w0, channel_multiplier=1,
            pattern=[[-1, inw]],
        )

    pt = ppool.tile([M, NTS], F32, tag="psum")
    for s in range(S):
        nc.tensor.matmul(
            pt[:, :nd],
            lhsT=band_i[:, s * BANDW:s * BANDW + M],
            rhs=xt[:, s:s + nd],
            start=(s == 0), stop=(s == S - 1),
        )

    # PSUM reads from a nonzero base partition are limited to 32 partitions,
    # so evacuate all M rows (above-diagonal rows come out 0 via the mask).
    m0 = 0
    # Columns q > pmax are entirely above the diagonal: skip them.
    wn = min(nd, pmax - q0 + 1)
    ot = opool.tile([128, NTS], F32, tag="out")
    if out_mask:
        # Evacuate PSUM through the causal staircase: keep iff
        # (p0+m) >= (q0+n) <=> stair[m, n + STAIR_C - d] with d = p0-q0.
        u0 = STAIR_C - d
        nc.vector.tensor_mul(
            ot[m0:M, :wn], pt[m0:M, :wn], stair[m0:M, u0:u0 + wn],
        )
    else:
        nc.any.tensor_copy(ot[m0:M, :wn], pt[m0:M, :wn])
    nc.sync.dma_start(
        out=y[i, p0 + m0:p0 + M, q0:q0 + wn],
        in_=ot[m0:M, :wn],
    )


def _build_bands(weight):
    """Host-side: per-image banded Toeplitz weights.
    bands[img, k, s*BANDW + m] = w[c(img), k-m, s] for k-m in [0, R)."""
    nimg = N * C
    bands = np.zeros((nimg, 128, S * BANDW), np.float32)
    m = np.arange(BANDW)
    for s in range(S):
        for r in range(R):
            # band[m+r, s*BANDW+m] = w[c, r, s]
            valid = m + r < 128
            mv = m[valid]
            for img in range(nimg):
                c = img % C
                bands[img, mv + r, s * BANDW + mv] = weight[c, r, s]
    return bands


def kernel(x, weight):
    x = np.ascontiguousarray(np.asarray(x, dtype=np.float32))
    weight = np.asarray(weight, dtype=np.float32)
    assert x.shape == (N, C, H, W) and weight.shape == (C, R, S)

    if "nc" not in _NC_CACHE:
        _NC_CACHE["nc"] = _build_program()
    nc = _NC_CACHE["nc"]

    x_imgs = x.reshape(N * C, H, W)
    bands = _build_bands(weight)
    in_maps = [
        {
            "x": x_imgs[k * IPC:(k + 1) * IPC],
            "bands": bands[k * IPC:(k + 1) * IPC],
        }
        for k in range(NCORES)
    ]
    res = run_bass_kernel_spmd(nc, in_maps, list(range(NCORES)))
    out = np.concatenate([res.results[k]["y"] for k in range(NCORES)], axis=0)
    return out.reshape(N, C, H, W)


# revision 7
# speedup vs baseline: 1.2632x; 1.2632x over previous
# Depthwise causal conv2d (N=2, C=16, H=W=2048, kernel 6x11) on 8 TRN2 cores.
#
# y[b,c,p,q] = sum_{r,s} w[c,r,s] * xm[b,c, p+r-5, q+s-5], xm = tril-masked x,
# y tril-masked.  Sharding: the 32 (b,c) images are independent; 4 per core.
#
# Per-core compute: for each output tile of [M<=123 rows, Nd<=512 cols], the
# row-tap contraction is a banded-Toeplitz matmul: for each of the S=11
# column taps, out[m, n] += band_s[k, m] * x[k, n+s] where band_s[k, m] =
# w[c, k-m, s] (k-m in [0,6)).  11 accumulating matmuls per tile into one
# PSUM bank.  Tiles fully above the causal diagonal are never computed or
# written (output DRAM is pre-zeroed); tiles crossing it get the causal mask
# applied to the input (gpsimd affine_select in SBUF) and to the output
# (DVE multiply with a staircase 0/1 tile during PSUM evacuation).
import sys

sys.path.insert(0, "/opt/trn_rl_repo")

import numpy as np

import concourse.bacc as bacc
import concourse.mybir as mybir
import concourse.tile as tile
from concourse.bass_utils import run_bass_kernel_spmd

N, C, H, W = 2, 16, 2048, 2048
R, S, PH, PW = 6, 11, 5, 5
NCORES = 8
IPC = (N * C) // NCORES  # images per core
MT = 123  # output rows per row-tile (128 input rows incl. 5-row halo)
NTS = 512  # max output cols per tile (one PSUM bank of fp32)
BANDW = 128  # allocated band width (cols used: M)
STAIR_W = 1536  # staircase mask width
STAIR_C = 511  # staircase offset: stair[i, u] = 1 iff u <= i + STAIR_C
F32 = mybir.dt.float32

# Matmul input dtype: "bf16" (fast weight load, half the input DRAM traffic,
# rel err ~2e-3) or "f32r" (full fp32 inputs, rel err ~2e-4).
DTYPE_MODE = "bf16"

_NC_CACHE = {}


def _xdt():
    return mybir.dt.bfloat16 if DTYPE_MODE == "bf16" else mybir.dt.float32r


def _np_xdt():
    if DTYPE_MODE == "bf16":
        import ml_dtypes

        return np.dtype(ml_dtypes.bfloat16)
    return np.dtype(np.float32)


def _row_tiles():
    out = []
    p0 = 0
    while p0 < H:
        out.append((p0, min(MT, H - p0)))
        p0 += MT
    return out


def _col_tiles(pmax):
    """Column tiles covering q in [0, pmax]; width 512 except the last,
    which shrinks to a multiple of 128 (>=256 keeps fp32r at full rate)."""
    out = []
    q0 = 0
    while q0 <= pmax:
        needed = pmax - q0 + 1
        if needed >= NTS:
            nd = NTS
        else:
            nd = min(NTS, max(256, 128 * ((needed + 127) // 128)))
        out.append((q0, nd))
        q0 += nd
    return out


def _build_program(rep=1):
    """One SPMD program: conv of IPC images [H, W] with per-image bands.

    rep > 1 wraps the whole body in a hardware loop executing it `rep`
    times — benchmarking only (amplifies kernel time above the fixed
    dispatch overhead of the execution path)."""
    import contextlib

    xdt = _xdt()
    nc = bacc.Bacc("TRN2", target_bir_lowering=False, debug=False,
                   num_devices=NCORES)
    x = nc.dram_tensor("x", [IPC, H, W], xdt, kind="ExternalInput")
    bands = nc.dram_tensor("bands", [IPC, 128, S * BANDW], xdt,
                           kind="ExternalInput")
    y = nc.dram_tensor("y", [IPC, H, W], F32, kind="ExternalOutput")

    row_tiles = _row_tiles()

    with tile.TileContext(nc) as tc:
        with (
            tc.tile_pool(name="const", bufs=1) as cpool,
            tc.tile_pool(name="xin", bufs=6) as xpool,
            tc.tile_pool(name="out", bufs=4) as opool,
            tc.tile_pool(name="psum", bufs=8, space="PSUM") as ppool,
            tc.For_i(0, rep, 1) if rep > 1 else contextlib.nullcontext(),
        ):
            # Per-image Toeplitz bands, resident for the whole kernel.
            bt = cpool.tile([128, IPC * S * BANDW], xdt)
            for i in range(IPC):
                nc.sync.dma_start(
                    out=bt[:, i * S * BANDW:(i + 1) * S * BANDW],
                    in_=bands[i],
                )
            # Staircase causal mask: stair[i, u] = 1 iff i + STAIR_C - u >= 0.
            stair = cpool.tile([128, STAIR_W], F32)
            nc.gpsimd.memset(stair[:], 1.0)
            nc.gpsimd.affine_select(
                out=stair[:], in_=stair[:],
                compare_op=mybir.AluOpType.is_ge, fill=0.0,
                base=STAIR_C, channel_multiplier=1,
                pattern=[[-1, STAIR_W]],
            )

            for i in range(IPC):
                band_i = bt[:, i * S * BANDW:(i + 1) * S * BANDW]
                for (p0, M) in row_tiles:
                    pmax = p0 + M - 1
                    for (q0, nd) in _col_tiles(pmax):
                        _emit_tile(nc, tc, xpool, opool, ppool, x, y, band_i,
                                   stair, i, p0, M, pmax, q0, nd)
    nc.compile()
    return nc


def _emit_tile(nc, tc, xpool, opool, ppool, x, y, band_i, stair,
               i, p0, M, pmax, q0, nd):
    xdt = _xdt()
    inw = nd + S - 1  # input tile width incl. halo
    # Input rows [p0-PH, p0+M), cols [q0-PW, q0+nd+PW) clipped to the image
    # and (on the right) to the causal extent pmax.
    h0 = p0 - PH
    hv0, hv1 = max(0, h0), min(H, p0 + M)
    w0 = q0 - PW
    wv0 = max(0, w0)
    wv1 = min(W, q0 + nd + PW, pmax + 1)

    d = p0 - q0  # diagonal offset of this tile
    # Causal mask on input needed iff the loaded region touches h < w.
    in_mask = h0 < wv1 - 1
    # Causal mask on output needed iff the tile crosses p < q.
    out_mask = p0 < q0 + nd - 1

    xt = xpool.tile([128, inw], xdt, tag="xin")
    nc.sync.dma_start(
        out=xt[hv0 - h0:hv1 - h0, wv0 - w0:wv1 - w0],
        in_=x[i, hv0:hv1, wv0:wv1],
    )
    if q0 == 0:
        # w in [-5, 0) is zero padding that the causal select keeps
        # (memset can't write fp32r): keep iff j >= PW.
        nc.gpsimd.affine_select(
            out=xt[:], in_=xt[:],
            compare_op=mybir.AluOpType.is_ge, fill=0.0,
            base=-PW, channel_multiplier=0,
            pattern=[[1, inw]],
        )
    if in_mask:
        # keep iff (h0 + k) >= (w0 + j)  <=>  k - j + (h0 - w0) >= 0.
        # Also zeroes the unloaded right-of-pmax region and, for p0 == 0,
        # the k < PH halo rows above the image (h < 0 keeps nothing).
        nc.gpsimd.affine_select(
            out=xt[:], in_=xt[:],
            compare_op=mybir.AluOpType.is_ge, fill=0.0,
            base=h0 - w0, channel_multiplier=1,
            pattern=[[-1, inw]],
        )

    pt = ppool.tile([M, NTS], F32, tag="psum")
    for s in range(S):
        nc.tensor.matmul(
            pt[:, :nd],
            lhsT=band_i[:, s * BANDW:s * BANDW + M],
            rhs=xt[:, s:s + nd],
            start=(s == 0), stop=(s == S - 1),
        )

    # Columns q > pmax are entirely above the diagonal: skip them.
    wn = min(nd, pmax - q0 + 1)
    ot = opool.tile([128, NTS], F32, tag="out")
    if out_mask:
        # Evacuate PSUM through the causal staircase: keep iff
        # (p0+m) >= (q0+n) <=> stair[m, n + STAIR_C - d] with d = p0-q0.
        u0 = STAIR_C - d
        nc.vector.tensor_mul(
            ot[:M, :wn], pt[:M, :wn], stair[:M, u0:u0 + wn],
        )
    else:
        nc.any.tensor_copy(ot[:M, :wn], pt[:M, :wn])
    nc.sync.dma_start(
        out=y[i, p0:p0 + M, q0:q0 + wn],
        in_=ot[:M, :wn],
    )


def _build_bands(weight):
    """Host-side: per-image banded Toeplitz weights.
    bands[img, k, s*BANDW + m] = w[c(img), k-m, s] for k-m in [0, R)."""
    nimg = N * C
    bands = np.zeros((nimg, 128, S * BANDW), np.float32)
    m = np.arange(BANDW)
    for s in range(S):
        for r in range(R):
            # band[m+r, s*BANDW+m] = w[c, r, s]
            valid = m + r < 128
            mv = m[valid]
            for img in range(nimg):
                c = img % C
                bands[img, mv + r, s * BANDW + mv] = weight[c, r, s]
    return bands.astype(_np_xdt())


def kernel(x, weight):
    x = np.asarray(x, dtype=np.float32)
    weight = np.asarray(weight, dtype=np.float32)
    assert x.shape == (N, C, H, W) and weight.shape == (C, R, S)

    if "nc" not in _NC_CACHE:
        _NC_CACHE["nc"] = _build_program()
    nc = _NC_CACHE["nc"]

    x_imgs = np.ascontiguousarray(x.reshape(N * C, H, W)).astype(
        _np_xdt(), copy=False)
    bands = _build_bands(weight)
    in_maps = [
        {
            "x": x_imgs[k * IPC:(k + 1) * IPC],
            "bands": bands[k * IPC:(k + 1) * IPC],
        }
        for k in range(NCORES)
    ]
    res = run_bass_kernel_spmd(nc, in_maps, list(range(NCORES)))
    out = np.concatenate([res.results[k]["y"] for k in range(NCORES)], axis=0)
    return out.reshape(N, C, H, W)


# revision 10
# speedup vs baseline: 1.2706x; 1.0058x over previous
# Depthwise causal conv2d (N=2, C=16, H=W=2048, kernel 6x11) on 8 TRN2 cores.
#
# y[b,c,p,q] = sum_{r,s} w[c,r,s] * xm[b,c, p+r-5, q+s-5], xm = tril-masked x,
# y tril-masked.  Sharding: the 32 (b,c) images are independent; 4 per core.
#
# Per-core compute: for each output tile of [M<=123 rows, Nd<=512 cols], the
# row-tap contraction is a banded-Toeplitz matmul: for each of the S=11
# column taps, out[m, n] += band_s[k, m] * x[k, n+s] where band_s[k, m] =
# w[c, k-m, s] (k-m in [0,6)).  11 accumulating matmuls per tile into one
# PSUM bank.  Tiles fully above the causal diagonal are never computed or
# written (output DRAM is pre-zeroed); tiles crossing it get the causal mask
# applied to the input (gpsimd affine_select in SBUF) and to the output
# (DVE multiply with a staircase 0/1 tile during PSUM evacuation).
import sys

sys.path.insert(0, "/opt/trn_rl_repo")

import numpy as np

import concourse.bacc as bacc
import concourse.mybir as mybir
import concourse.tile as tile
from concourse.bass_utils import run_bass_kernel_spmd

N, C, H, W = 2, 16, 2048, 2048
R, S, PH, PW = 6, 11, 5, 5
NCORES = 8
IPC = (N * C) // NCORES  # images per core
MT = 123  # output rows per row-tile (128 input rows incl. 5-row halo)
NTS = 512  # max output cols per tile (one PSUM bank of fp32)
BANDW = 128  # allocated band width (cols used: M)
STAIR_W = 1536  # staircase mask width
STAIR_C = 511  # staircase offset: stair[i, u] = 1 iff u <= i + STAIR_C
F32 = mybir.dt.float32

# Matmul input dtype: "f16"/"bf16" (fast weight load, half the input DRAM
# traffic; f16 has 10 mantissa bits vs bf16's 7 at the same matmul rate)
# or "f32r" (full fp32 inputs, rel err ~2e-4, ~1.5x slower).
DTYPE_MODE = "f16"

_NC_CACHE = {}


def _xdt():
    return {
        "f16": mybir.dt.float16,
        "bf16": mybir.dt.bfloat16,
        "f32r": mybir.dt.float32r,
    }[DTYPE_MODE]


def _np_xdt():
    if DTYPE_MODE == "f16":
        return np.dtype(np.float16)
    if DTYPE_MODE == "bf16":
        import ml_dtypes

        return np.dtype(ml_dtypes.bfloat16)
    return np.dtype(np.float32)


def _row_tiles():
    out = []
    p0 = 0
    while p0 < H:
        out.append((p0, min(MT, H - p0)))
        p0 += MT
    return out


def _col_tiles(pmax):
    """Column tiles covering q in [0, pmax]; width 512 except the last,
    which shrinks to a multiple of 128 (fp32r needs >=256 for full rate)."""
    min_nd = 256 if DTYPE_MODE == "f32r" else 128
    out = []
    q0 = 0
    while q0 <= pmax:
        needed = pmax - q0 + 1
        if needed >= NTS:
            nd = NTS
        else:
            nd = min(NTS, max(min_nd, 128 * ((needed + 127) // 128)))
        out.append((q0, nd))
        q0 += nd
    return out


def _build_program(rep=1):
    """One SPMD program: conv of IPC images [H, W] with per-image bands.

    rep > 1 wraps the whole body in a hardware loop executing it `rep`
    times — benchmarking only (amplifies kernel time above the fixed
    dispatch overhead of the execution path)."""
    import contextlib

    xdt = _xdt()
    nc = bacc.Bacc("TRN2", target_bir_lowering=False, debug=False,
                   num_devices=NCORES)
    x = nc.dram_tensor("x", [IPC, H, W], xdt, kind="ExternalInput")
    bands = nc.dram_tensor("bands", [IPC, 128, S * BANDW], xdt,
                           kind="ExternalInput")
    y = nc.dram_tensor("y", [IPC, H, W], F32, kind="ExternalOutput")

    row_tiles = _row_tiles()

    with tile.TileContext(nc) as tc:
        with (
            tc.tile_pool(name="const", bufs=1) as cpool,
            tc.tile_pool(name="xin", bufs=6) as xpool,
            tc.tile_pool(name="out", bufs=4) as opool,
            tc.tile_pool(name="psum", bufs=8, space="PSUM") as ppool,
            tc.For_i(0, rep, 1) if rep > 1 else contextlib.nullcontext(),
        ):
            # Per-image Toeplitz bands, resident for the whole kernel.
            bt = cpool.tile([128, IPC * S * BANDW], xdt)
            for i in range(IPC):
                nc.sync.dma_start(
                    out=bt[:, i * S * BANDW:(i + 1) * S * BANDW],
                    in_=bands[i],
                )
            # Staircase causal mask: stair[i, u] = 1 iff i + STAIR_C - u >= 0.
            stair = cpool.tile([128, STAIR_W], F32)
            nc.gpsimd.memset(stair[:], 1.0)
            nc.gpsimd.affine_select(
                out=stair[:], in_=stair[:],
                compare_op=mybir.AluOpType.is_ge, fill=0.0,
                base=STAIR_C, channel_multiplier=1,
                pattern=[[-1, STAIR_W]],
            )

            for i in range(IPC):
                band_i = bt[:, i * S * BANDW:(i + 1) * S * BANDW]
                for (p0, M) in row_tiles:
                    pmax = p0 + M - 1
                    for (q0, nd) in _col_tiles(pmax):
                        _emit_tile(nc, tc, xpool, opool, ppool, x, y, band_i,
                                   stair, i, p0, M, pmax, q0, nd)
    nc.compile()
    return nc


def _emit_tile(nc, tc, xpool, opool, ppool, x, y, band_i, stair,
               i, p0, M, pmax, q0, nd):
    xdt = _xdt()
    inw = nd + S - 1  # input tile width incl. halo
    # Input rows [p0-PH, p0+M), cols [q0-PW, q0+nd+PW) clipped to the image
    # and (on the right) to the causal extent pmax.
    h0 = p0 - PH
    hv0, hv1 = max(0, h0), min(H, p0 + M)
    w0 = q0 - PW
    wv0 = max(0, w0)
    wv1 = min(W, q0 + nd + PW, pmax + 1)

    d = p0 - q0  # diagonal offset of this tile
    # Causal mask on input needed iff the loaded region touches h < w.
    in_mask = h0 < wv1 - 1
    # Causal mask on output needed iff the tile crosses p < q.
    out_mask = p0 < q0 + nd - 1

    xt = xpool.tile([128, inw], xdt, tag="xin")
    nc.sync.dma_start(
        out=xt[hv0 - h0:hv1 - h0, wv0 - w0:wv1 - w0],
        in_=x[i, hv0:hv1, wv0:wv1],
    )
    if q0 == 0:
        # w in [-5, 0) is zero padding that the causal select keeps
        # (memset can't write fp32r): keep iff j >= PW.
        nc.gpsimd.affine_select(
            out=xt[:], in_=xt[:],
            compare_op=mybir.AluOpType.is_ge, fill=0.0,
            base=-PW, channel_multiplier=0,
            pattern=[[1, inw]],
        )
    if in_mask:
        # keep iff (h0 + k) >= (w0 + j)  <=>  k - j + (h0 - w0) >= 0.
        # Also zeroes the unloaded right-of-pmax region and, for p0 == 0,
        # the k < PH halo rows above the image (h < 0 keeps nothing).
        nc.gpsimd.affine_select(
            out=xt[:], in_=xt[:],
            compare_op=mybir.AluOpType.is_ge, fill=0.0,
            base=h0 - w0, channel_multiplier=1,
            pattern=[[-1, inw]],
        )

    pt = ppool.tile([M, NTS], F32, tag="psum")
    for s in range(S):
        nc.tensor.matmul(
            pt[:, :nd],
            lhsT=band_i[:, s * BANDW:s * BANDW + M],
            rhs=xt[:, s:s + nd],
            start=(s == 0), stop=(s == S - 1),
        )

    # Columns q > pmax are entirely above the diagonal: skip them.
    wn = min(nd, pmax - q0 + 1)
    ot = opool.tile([128, NTS], F32, tag="out")
    if out_mask:
        # Evacuate PSUM through the causal staircase: keep iff
        # (p0+m) >= (q0+n) <=> stair[m, n + STAIR_C - d] with d = p0-q0.
        u0 = STAIR_C - d
        nc.vector.tensor_mul(
            ot[:M, :wn], pt[:M, :wn], stair[:M, u0:u0 + wn],
        )
    else:
        nc.any.tensor_copy(ot[:M, :wn], pt[:M, :wn])
    nc.sync.dma_start(
        out=y[i, p0:p0 + M, q0:q0 + wn],
        in_=ot[:M, :wn],
    )


def _build_bands(weight):
    """Host-side: per-image banded Toeplitz weights.
    bands[img, k, s*BANDW + m] = w[c(img), k-m, s] for k-m in [0, R)."""
    nimg = N * C
    bands = np.zeros((nimg, 128, S * BANDW), np.float32)
    m = np.arange(BANDW)
    for s in range(S):
        for r in range(R):
            # band[m+r, s*BANDW+m] = w[c, r, s]
            valid = m + r < 128
            mv = m[valid]
            for img in range(nimg):
                c = img % C
                bands[img, mv + r, s * BANDW + mv] = weight[c, r, s]
    return bands.astype(_np_xdt())


def kernel(x, weight):
    x = np.asarray(x, dtype=np.float32)
    weight = np.asarray(weight, dtype=np.float32)
    assert x.shape == (N, C, H, W) and weight.shape == (C, R, S)

    if "nc" not in _NC_CACHE:
        _NC_CACHE["nc"] = _build_program()
    nc = _NC_CACHE["nc"]

    x_imgs = np.ascontiguousarray(x.reshape(N * C, H, W)).astype(
        _np_xdt(), copy=False)
    bands = _build_bands(weight)
    in_maps = [
        {
            "x": x_imgs[k * IPC:(k + 1) * IPC],
            "bands": bands[k * IPC:(k + 1) * IPC],
        }
        for k in range(NCORES)
    ]
    res = run_bass_kernel_spmd(nc, in_maps, list(range(NCORES)))
    out = np.concatenate([res.results[k]["y"] for k in range(NCORES)], axis=0)
    return out.reshape(N, C, H, W)
